# revision 20
# baseline (speedup 1.0000x reference)
"""Trainium2 Bass kernel: 2-layer GAT (nn_GAT_1709396983866).

Strategy (graph/data parallel over 8 NeuronCores):
  * Nodes are permuted and packed into blocks of 128 ("positions"); blocks are
    balanced by in-degree so every block has <= CMAX*128 incident edges.
  * Each core owns NBLK/8 blocks: it computes segment-softmax attention and the
    scatter-add aggregation for the destination nodes of its blocks.
  * Edge work is done 128 edges at a time ("chunks") with edges on SBUF
    partitions. Per-destination segment ops (softmax denominator + weighted
    aggregation) are done with a one-hot mask matmul on the tensor engine:
        mask[e, d] = (dst_local[e] == d)  -> acc[d, :] += mask.T @ msg[e, :]
    where msg = [h_src * exp(leaky(logit)) | exp(leaky(logit))].
  * Node features for gathers live in DRAM tables:
        HAUG  [NPAD, 144] bf16 = [adst(8) | asrc(8) | h(128)]        (layer 1)
        HAUG2 [NPAD,  48] bf16 = [adst2(1) | asrc2(1) | h2(40) | pad] (layer 2)
    HAUG is computed replicated on every core (cheaper than AllGather);
    HAUG2 shards are computed by the owning core and AllGathered.
  * exp(leaky_relu(x)) == max(exp(x), exp(0.2 x)) exactly (monotonicity), so
    no branch is needed; segment-max subtraction is skipped (logits are O(1),
    softmax is shift-invariant so results match the reference to fp32 noise).

kernel(**inputs) takes the full unsharded inputs and returns the full output.
"""

import os
import numpy as np
import ml_dtypes

import concourse.bass as bass
import concourse.tile as tile
from concourse import mybir
from concourse.bass_utils import run_bass_kernel_spmd
from concourse.tile_rust import add_dep_helper


# Per-opcode embedded sync-wait slot budget in walrus codegen (empirical).
# Excess waits are hoisted onto standalone EventSemaphore instructions (the
# same form nc.<engine>.wait_ge emits), one wait each.
_WAIT_LIMITS = {}
_WAIT_LIMIT_DEFAULT = 1
_NOSPLIT_OPS = ("EventSemaphore",)


def _split_excess_waits(nc):
    """Move excess sem waits onto preceding same-engine wait instructions.

    The NX sequencer executes an instruction's wait commands in stream order
    before dispatching it, so hoisting leading waits onto EventSemaphore
    instructions immediately before it is semantically identical — and keeps
    every instruction within walrus codegen's per-opcode wait-slot limits.
    """
    nid = [0]

    def mk_wait(engine, wait):
        nid[0] += 1
        ev = mybir.InstEventSemaphore(
            name=f"waitsplit-{nid[0]}", ins=[], outs=[])
        ev.engine = engine
        ev.sync_info = mybir.SyncInfo(on_wait=[wait], on_update=[])
        return ev

    for fn in nc.m.functions:
        for bb in fn.blocks:
            out = []
            for inst in bb.instructions:
                si = inst.sync_info
                waits = list(si.on_wait) if si and si.on_wait else []
                lim = _WAIT_LIMITS.get(inst.opcode, _WAIT_LIMIT_DEFAULT)
                if len(waits) > lim and inst.opcode not in _NOSPLIT_OPS:
                    excess, keep = waits[:-lim], waits[-lim:]
                    for w in excess:
                        out.append(mk_wait(inst.engine, w))
                    inst.sync_info = mybir.SyncInfo(
                        on_wait=keep, on_update=list(si.on_update or []))
                out.append(inst)
            bb.instructions = out


def _phase_barrier(tc, nc):
    """All-engine barrier that also advances every engine's observed DMA-lane
    clocks. A bare strict barrier syncs through one SP nop, but Tile's wait
    emission is not transitive, so the first DMA per engine after it would
    still carry one wait per DMA-lane semaphore — more than the DIRECT2D
    codegen wait-slot limit. Give each engine its own wait-soaking nop first.
    """
    curr_bb = nc.cur_bb
    prev = list(curr_bb.bb.instructions)
    for eng in (nc.gpsimd, nc.sync, nc.scalar, nc.vector, nc.tensor):
        nop = eng.nop()
        for inst in prev:
            add_dep_helper(
                nop.ins, inst,
                sync=bass.sync_unless_reorderable_target(
                    inst, inst.is_executable()),
                reason="phase-barrier soak")
    tc.strict_bb_all_engine_barrier()

# -------- problem constants (hardcoded, per spec) --------
N_NODES = 100000
IN_DIM = 128
HID = 128
OUT_DIM = 40
H1 = 8
C1 = 16
NEG_SLOPE = 0.2
EPS = 1e-16
N_CORES = 8
P = 128

F32 = mybir.dt.float32
BF16 = mybir.dt.bfloat16
I32 = mybir.dt.int32
AF = mybir.ActivationFunctionType
OP = mybir.AluOpType

PAD_DLOC = 999.0  # dst_local value for padding edge slots (matches no lane)
DENOM_FLOOR = 1e-6  # real nodes always have denom >= exp(-|logit|) >> this

# tiles per batch in the HAUG generation phase
TPB = 4


def _bd(*shape):
    return np.zeros(shape, np.float32)


def build_program(bpc, cmax, npad, n_cores):
    """Build the Bass program. Returns nc.

    bpc: blocks per core; cmax: chunks (of 128 edge slots) per block;
    npad: padded node-position count == bpc * n_cores * 128.
    """
    nblk = bpc * n_cores
    assert npad == nblk * P
    assert nblk % TPB == 0
    HGW = 2 * H1 + HID            # 144: [adst(8)|asrc(8)|h(128)]
    HG2W = 48                     # [adst2|asrc2|h2(40)|pad(6)]
    ACC1W = HID + H1              # 136: [agg(128)|denom(8)]
    ACC2W = OUT_DIM + 1           # 41

    nc = bass.Bass(num_devices=n_cores)

    # ---------------- I/O ----------------
    XT = nc.dram_tensor("XT", [IN_DIM, npad], F32, kind="ExternalInput")
    W1d = nc.dram_tensor("W1", [IN_DIM, HID], F32, kind="ExternalInput")
    W1Td = nc.dram_tensor("W1T", [HID, IN_DIM], F32, kind="ExternalInput")
    A1BDd = nc.dram_tensor("A1BD", [HID, 2 * H1], F32, kind="ExternalInput")
    W2d = nc.dram_tensor("W2", [HID, OUT_DIM], F32, kind="ExternalInput")
    W2Td = nc.dram_tensor("W2T", [OUT_DIM, HID], F32, kind="ExternalInput")
    A2Td = nc.dram_tensor("A2T", [OUT_DIM, 2], F32, kind="ExternalInput")
    B1Rd = nc.dram_tensor("B1R", [P, HID], F32, kind="ExternalInput")
    B2Rd = nc.dram_tensor("B2R", [P, OUT_DIM], F32, kind="ExternalInput")
    IOTAd = nc.dram_tensor("IOTA", [P, P], F32, kind="ExternalInput")
    IDENTd = nc.dram_tensor("IDENT", [P, P], BF16, kind="ExternalInput")
    SRCd = nc.dram_tensor("SRC", [P, bpc * cmax], I32, kind="ExternalInput")
    DSTGd = nc.dram_tensor("DSTG", [P, bpc * cmax], I32, kind="ExternalInput")
    DLOCd = nc.dram_tensor("DLOC", [P, bpc * cmax], F32, kind="ExternalInput")
    OUTd = nc.dram_tensor("OUT", [bpc * P, OUT_DIM], F32, kind="ExternalOutput")

    # ---------------- internal DRAM ----------------
    HAUGd = nc.dram_tensor("HAUG", [npad, HGW], BF16)
    HG2Ld = nc.dram_tensor("HG2L", [bpc * P, HG2W], BF16)
    HG2d = nc.dram_tensor("HG2", [npad, HG2W], BF16, addr_space="Shared")

    with tile.TileContext(nc) as tc:
        with tc.tile_pool(name="consts", bufs=1) as cp, \
             tc.tile_pool(name="pa", bufs=3) as pa:
            # ---- load constants / weights (bf16 where used by PE) ----
            W1_sb = cp.tile([IN_DIM, HID], BF16)
            nc.gpsimd.dma_start(out=W1_sb[:], in_=W1d[:, :])
            W1T_sb = cp.tile([HID, IN_DIM], BF16)
            nc.gpsimd.dma_start(out=W1T_sb[:], in_=W1Td[:, :])
            A1BD_sb = cp.tile([HID, 2 * H1], BF16)
            nc.gpsimd.dma_start(out=A1BD_sb[:], in_=A1BDd[:, :])
            W2T_sb = cp.tile([OUT_DIM, HID], BF16)
            nc.gpsimd.dma_start(out=W2T_sb[:], in_=W2Td[:, :])
            A2T_sb = cp.tile([OUT_DIM, 2], BF16)
            nc.gpsimd.dma_start(out=A2T_sb[:], in_=A2Td[:, :])
            B1R_sb = cp.tile([P, HID], F32)
            nc.sync.dma_start(out=B1R_sb[:], in_=B1Rd[:, :])
            B2R_sb = cp.tile([P, OUT_DIM], F32)
            nc.sync.dma_start(out=B2R_sb[:], in_=B2Rd[:, :])
            IOTA_sb = cp.tile([P, P], F32)
            nc.sync.dma_start(out=IOTA_sb[:], in_=IOTAd[:, :])
            IDENT_sb = cp.tile([P, P], BF16)
            nc.sync.dma_start(out=IDENT_sb[:], in_=IDENTd[:, :])

            # WA = W1 @ A1BD; W2AUG = [w_dst2 | w_src2 | W2]
            WA_sb = cp.tile([IN_DIM, 2 * H1], BF16)
            W2AUG_sb = cp.tile([HID, 2 + OUT_DIM], BF16)
            with tc.tile_pool(name="cpsum", bufs=1, space="PSUM") as cps:
                WA_ps = cps.tile([IN_DIM, 2 * H1], F32)
                nc.tensor.matmul(WA_ps[:], lhsT=W1T_sb[:], rhs=A1BD_sb[:],
                                 start=True, stop=True)
                nc.vector.tensor_copy(out=WA_sb[:], in_=WA_ps[:])
                wt_ps = cps.tile([HID, 2], F32)
                nc.tensor.matmul(wt_ps[:], lhsT=W2T_sb[:], rhs=A2T_sb[:],
                                 start=True, stop=True)
                nc.vector.tensor_copy(out=W2AUG_sb[:, 0:2], in_=wt_ps[:])
            nc.gpsimd.dma_start(out=W2AUG_sb[:, 2:2 + OUT_DIM], in_=W2d[:, :])

            # ================= Phase A: HAUG generation (replicated) ========
            # (pa is opened at the top level and stays open: letting its SBUF
            # addresses be reused would add one WAW wait per DMA lane to the
            # first phase-B gather -> DIRECT2D wait-slot limit.)
            with tc.tile_pool(name="papsum", bufs=2, space="PSUM") as pap:
                for tb in range(nblk // TPB):
                    c0 = tb * TPB * P
                    xt = pa.tile([IN_DIM, TPB * P], BF16, tag="xt")
                    nc.gpsimd.dma_start(out=xt[:], in_=XT[:, c0:c0 + TPB * P])
                    h_ps = pap.tile([P, TPB * HID], F32, tag="h_ps")
                    al_ps = pap.tile([P, TPB * 2 * H1], F32, tag="al_ps")
                    for i in range(TPB):
                        lhs = xt[:, i * P:(i + 1) * P]
                        nc.tensor.matmul(h_ps[:, i * HID:(i + 1) * HID],
                                         lhsT=lhs, rhs=W1_sb[:],
                                         start=True, stop=True)
                        nc.tensor.matmul(al_ps[:, i * 2 * H1:(i + 1) * 2 * H1],
                                         lhsT=lhs, rhs=WA_sb[:],
                                         start=True, stop=True)
                    hg = pa.tile([P, TPB * HGW], BF16, tag="hg")
                    hg3 = hg[:].rearrange("p (t c) -> p t c", c=HGW)
                    h3 = h_ps[:].rearrange("p (t c) -> p t c", c=HID)
                    al3 = al_ps[:].rearrange("p (t c) -> p t c", c=2 * H1)
                    nc.vector.tensor_copy(out=hg3[:, :, 2 * H1:HGW], in_=h3)
                    nc.vector.tensor_copy(out=hg3[:, :, 0:2 * H1], in_=al3)
                    for i in range(TPB):
                        nc.sync.dma_start(
                            out=HAUGd[c0 + i * P:c0 + (i + 1) * P, :],
                            in_=hg3[:, i, :])

            # ---- persistent edge metadata (loaded once, never reused) ----
            srcT_all = cp.tile([P, bpc * cmax], I32)
            dstgT_all = cp.tile([P, bpc * cmax], I32)
            dlocT_all = cp.tile([P, bpc * cmax], F32)
            nc.sync.dma_start(out=srcT_all[:], in_=SRCd[:, :])
            nc.sync.dma_start(out=dstgT_all[:], in_=DSTGd[:, :])
            nc.sync.dma_start(out=dlocT_all[:], in_=DLOCd[:, :])

            # barrier between phases (true ordering comes from Tile's real
            # dependencies; excess waits are legalized by _split_excess_waits)
            tc.strict_bb_all_engine_barrier()

            # ================= Phase B: layer-1 edge processing =============
            with tc.tile_pool(name="pb", bufs=8) as pb, \
                 tc.tile_pool(name="pbe", bufs=2) as pbe, \
                 tc.tile_pool(name="pbpsum", bufs=2, space="PSUM") as pbp:
                for b in range(bpc):
                    srcT = srcT_all[:, b * cmax:(b + 1) * cmax]
                    dstgT = dstgT_all[:, b * cmax:(b + 1) * cmax]
                    dlocT = dlocT_all[:, b * cmax:(b + 1) * cmax]

                    acc = pbp.tile([P, ACC1W], F32, tag="acc")
                    for c in range(cmax):
                        g1 = pb.tile([P, HGW], BF16, tag="g1")
                        nc.gpsimd.indirect_dma_start(
                            out=g1[:], out_offset=None, in_=HAUGd[:, :],
                            in_offset=bass.IndirectOffsetOnAxis(
                                ap=srcT[:, c:c + 1], axis=0))
                        g2 = pb.tile([P, H1], BF16, tag="g2")
                        nc.gpsimd.indirect_dma_start(
                            out=g2[:], out_offset=None, in_=HAUGd[:, :],
                            in_offset=bass.IndirectOffsetOnAxis(
                                ap=dstgT[:, c:c + 1], axis=0))
                        mask = pb.tile([P, P], BF16, tag="mask")
                        nc.vector.tensor_scalar(
                            out=mask[:], in0=IOTA_sb[:],
                            scalar1=dlocT[:, c:c + 1], scalar2=None,
                            op0=OP.is_equal)
                        lg = pb.tile([P, H1], F32, tag="lg")
                        # logit = asrc[src] + adst[dst]
                        nc.vector.tensor_tensor(out=lg[:], in0=g1[:, H1:2 * H1],
                                                in1=g2[:], op=OP.add)
                        # exp(leaky_relu(x)) = max(exp(x), exp(0.2x))
                        e1 = pb.tile([P, H1], F32, tag="e1")
                        nc.scalar.activation(out=e1[:], in_=lg[:], func=AF.Exp)
                        e2 = pb.tile([P, H1], F32, tag="e2")
                        nc.scalar.activation(out=e2[:], in_=lg[:], func=AF.Exp,
                                             scale=NEG_SLOPE)
                        msg = pb.tile([P, ACC1W], BF16, tag="msg")
                        nc.vector.tensor_tensor(out=msg[:, HID:ACC1W],
                                                in0=e1[:], in1=e2[:], op=OP.max)
                        # msg[:, 0:128] = h_src * exp (broadcast over C1)
                        h_src3 = g1[:, 2 * H1:HGW].rearrange(
                            "p (h c) -> p h c", c=C1)
                        expb = msg[:, HID:ACC1W].unsqueeze(2).to_broadcast(
                            [P, H1, C1])
                        m3 = msg[:, 0:HID].rearrange("p (h c) -> p h c", c=C1)
                        nc.vector.tensor_tensor(out=m3, in0=h_src3, in1=expb,
                                                op=OP.mult)
                        nc.tensor.matmul(acc[:], lhsT=mask[:], rhs=msg[:],
                                         start=(c == 0), stop=(c == cmax - 1))

                    # ---- block epilogue: normalize, bias, ELU, haug2 ----
                    dinv = pbe.tile([P, H1], F32, tag="dinv")
                    nc.vector.tensor_scalar(
                        out=dinv[:], in0=acc[:, HID:ACC1W],
                        scalar1=EPS, scalar2=DENOM_FLOOR,
                        op0=OP.add, op1=OP.max)
                    nc.vector.reciprocal(out=dinv[:], in_=dinv[:])
                    h1 = pbe.tile([P, HID], F32, tag="h1")
                    a3 = acc[:, 0:HID].rearrange("p (h c) -> p h c", c=C1)
                    dv3 = dinv[:].unsqueeze(2).to_broadcast([P, H1, C1])
                    h13 = h1[:].rearrange("p (h c) -> p h c", c=C1)
                    nc.vector.tensor_tensor(out=h13, in0=a3, in1=dv3,
                                            op=OP.mult)
                    nc.vector.tensor_tensor(out=h1[:], in0=h1[:], in1=B1R_sb[:],
                                            op=OP.add)
                    # ELU(x) = max(x,0) + min(exp(x)-1, 0)
                    ex = pbe.tile([P, HID], F32, tag="ex")
                    nc.scalar.activation(out=ex[:], in_=h1[:], func=AF.Exp)
                    nc.vector.tensor_scalar(out=ex[:], in0=ex[:],
                                            scalar1=-1.0, scalar2=0.0,
                                            op0=OP.add, op1=OP.min)
                    h1r = pbe.tile([P, HID], F32, tag="h1r")
                    nc.vector.tensor_scalar(out=h1r[:], in0=h1[:],
                                            scalar1=0.0, scalar2=None,
                                            op0=OP.max)
                    h1e = pbe.tile([P, HID], BF16, tag="h1e")
                    nc.vector.tensor_tensor(out=h1e[:], in0=h1r[:], in1=ex[:],
                                            op=OP.add)
                    h1T_ps = pbp.tile([P, HID], BF16, tag="h1T_ps")
                    nc.tensor.transpose(h1T_ps[:], h1e[:], IDENT_sb[:])
                    h1T = pbe.tile([P, HID], BF16, tag="h1T")
                    nc.vector.tensor_copy(out=h1T[:], in_=h1T_ps[:])
                    hg2_ps = pbp.tile([P, 2 + OUT_DIM], F32, tag="hg2_ps")
                    nc.tensor.matmul(hg2_ps[:], lhsT=h1T[:], rhs=W2AUG_sb[:],
                                     start=True, stop=True)
                    hg2 = pbe.tile([P, HG2W], BF16, tag="hg2")
                    nc.vector.tensor_copy(out=hg2[:, 0:2 + OUT_DIM],
                                          in_=hg2_ps[:])
                    nc.gpsimd.memset(hg2[:, 2 + OUT_DIM:HG2W], 0.0)
                    nc.sync.dma_start(out=HG2Ld[b * P:(b + 1) * P, :],
                                      in_=hg2[:])

                # ============= AllGather of HAUG2 shards ===============
                nc.gpsimd.collective_compute(
                    "AllGather", OP.bypass,
                    replica_groups=[list(range(n_cores))],
                    ins=[HG2Ld[:, :].opt()],
                    outs=[HG2d[:, :].opt()],
                )

                # barrier before phase C
                tc.strict_bb_all_engine_barrier()

                # ============= Phase C: layer-2 edge processing =============
                pc, pce = pb, pbe
                pcp = pbp
                for b in range(bpc):
                    srcT = srcT_all[:, b * cmax:(b + 1) * cmax]
                    dstgT = dstgT_all[:, b * cmax:(b + 1) * cmax]
                    dlocT = dlocT_all[:, b * cmax:(b + 1) * cmax]

                    acc2 = pcp.tile([P, ACC2W], F32, tag="acc2")
                    for c in range(cmax):
                        g1 = pc.tile([P, HG2W], BF16, tag="g1")
                        nc.gpsimd.indirect_dma_start(
                            out=g1[:], out_offset=None, in_=HG2d[:, :],
                            in_offset=bass.IndirectOffsetOnAxis(
                                ap=srcT[:, c:c + 1], axis=0))
                        g2 = pc.tile([P, 1], BF16, tag="g2")
                        nc.gpsimd.indirect_dma_start(
                            out=g2[:], out_offset=None, in_=HG2d[:, :],
                            in_offset=bass.IndirectOffsetOnAxis(
                                ap=dstgT[:, c:c + 1], axis=0))
                        mask = pc.tile([P, P], BF16, tag="mask")
                        nc.vector.tensor_scalar(
                            out=mask[:], in0=IOTA_sb[:],
                            scalar1=dlocT[:, c:c + 1], scalar2=None,
                            op0=OP.is_equal)
                        lg = pc.tile([P, 1], F32, tag="lg")
                        nc.vector.tensor_tensor(out=lg[:], in0=g1[:, 1:2],
                                                in1=g2[:], op=OP.add)
                        e1 = pc.tile([P, 1], F32, tag="e1")
                        nc.scalar.activation(out=e1[:], in_=lg[:], func=AF.Exp)
                        e2 = pc.tile([P, 1], F32, tag="e2")
                        nc.scalar.activation(out=e2[:], in_=lg[:], func=AF.Exp,
                                             scale=NEG_SLOPE)
                        msg = pc.tile([P, ACC2W], BF16, tag="msg")
                        nc.vector.tensor_tensor(out=msg[:, OUT_DIM:ACC2W],
                                                in0=e1[:], in1=e2[:], op=OP.max)
                        expb = msg[:, OUT_DIM:ACC2W].to_broadcast([P, OUT_DIM])
                        nc.vector.tensor_tensor(out=msg[:, 0:OUT_DIM],
                                                in0=g1[:, 2:2 + OUT_DIM],
                                                in1=expb, op=OP.mult)
                        nc.tensor.matmul(acc2[:], lhsT=mask[:], rhs=msg[:],
                                         start=(c == 0), stop=(c == cmax - 1))

                    # ---- block epilogue: normalize, bias, log_softmax ----
                    dinv2 = pce.tile([P, 1], F32, tag="dinv2")
                    nc.vector.tensor_scalar(
                        out=dinv2[:], in0=acc2[:, OUT_DIM:ACC2W],
                        scalar1=EPS, scalar2=DENOM_FLOOR,
                        op0=OP.add, op1=OP.max)
                    nc.vector.reciprocal(out=dinv2[:], in_=dinv2[:])
                    o = pce.tile([P, OUT_DIM], F32, tag="o")
                    nc.vector.tensor_scalar(out=o[:], in0=acc2[:, 0:OUT_DIM],
                                            scalar1=dinv2[:, 0:1], scalar2=None,
                                            op0=OP.mult)
                    nc.vector.tensor_tensor(out=o[:], in0=o[:], in1=B2R_sb[:],
                                            op=OP.add)
                    nm = pce.tile([P, 1], F32, tag="nm")
                    nc.vector.tensor_reduce(out=nm[:], in_=o[:],
                                            axis=mybir.AxisListType.X,
                                            op=OP.max, negate=True)
                    e2t = pce.tile([P, OUT_DIM], F32, tag="e2t")
                    s2 = pce.tile([P, 1], F32, tag="s2")
                    nc.scalar.activation(out=e2t[:], in_=o[:], func=AF.Exp,
                                         bias=nm[:, 0:1], accum_out=s2[:, 0:1])
                    ls = pce.tile([P, 1], F32, tag="ls")
                    nc.scalar.activation(out=ls[:], in_=s2[:], func=AF.Ln)
                    sh = pce.tile([P, 1], F32, tag="sh")
                    nc.vector.tensor_tensor(out=sh[:], in0=nm[:], in1=ls[:],
                                            op=OP.subtract)
                    ot = pce.tile([P, OUT_DIM], F32, tag="ot")
                    nc.vector.tensor_scalar(out=ot[:], in0=o[:],
                                            scalar1=sh[:, 0:1], scalar2=None,
                                            op0=OP.add)
                    nc.sync.dma_start(out=OUTd[b * P:(b + 1) * P, :],
                                      in_=ot[:])

    return nc


# ----------------------------------------------------------------------------
# host-side preprocessing (pure index/layout work, no float math on data)
# ----------------------------------------------------------------------------

def preprocess_graph(src, dst, n_nodes, nblk):
    """Assign nodes to balanced blocks of 128; lay out edges per block.

    Returns (pos_of_node, SRCt, DSTGt, DLOCt, cmax).
    """
    E = src.shape[0]
    deg = np.bincount(dst, minlength=n_nodes)
    order = np.argsort(-deg, kind="stable")
    r = np.arange(n_nodes)
    rounds, posr = r // nblk, r % nblk
    binr = np.where(rounds % 2 == 0, posr, nblk - 1 - posr)
    blk_of_node = np.empty(n_nodes, np.int64)
    blk_of_node[order] = binr
    cnt = np.bincount(blk_of_node, minlength=nblk)
    assert cnt.max() <= P, f"block overfull: {cnt.max()}"
    node_sorted = np.argsort(blk_of_node, kind="stable")
    starts = np.concatenate([[0], np.cumsum(cnt)[:-1]])
    slot_sorted = np.arange(n_nodes) - np.repeat(starts, cnt)
    slot_of_node = np.empty(n_nodes, np.int64)
    slot_of_node[node_sorted] = slot_sorted
    pos_of_node = blk_of_node * P + slot_of_node

    eblk = blk_of_node[dst]
    ecnt = np.bincount(eblk, minlength=nblk)
    cmax = max(1, int(np.ceil(ecnt.max() / P)))
    cap = cmax * P
    eord = np.argsort(eblk, kind="stable")
    estarts = np.concatenate([[0], np.cumsum(ecnt)[:-1]])
    eslot = np.arange(E) - np.repeat(estarts, ecnt)
    b_ = eblk[eord]
    SRCa = np.zeros((nblk, cap), np.int32)
    DSTGa = np.zeros((nblk, cap), np.int32)
    DLOCa = np.full((nblk, cap), PAD_DLOC, np.float32)
    SRCa[b_, eslot] = pos_of_node[src[eord]]
    DSTGa[b_, eslot] = pos_of_node[dst[eord]]
    DLOCa[b_, eslot] = slot_of_node[dst[eord]]
    # [nblk, cap] -> [nblk, P, cmax]  (edge slot s = c*P + p)
    SRCt = np.ascontiguousarray(SRCa.reshape(nblk, cmax, P).transpose(0, 2, 1))
    DSTGt = np.ascontiguousarray(DSTGa.reshape(nblk, cmax, P).transpose(0, 2, 1))
    DLOCt = np.ascontiguousarray(DLOCa.reshape(nblk, cmax, P).transpose(0, 2, 1))
    return pos_of_node, SRCt, DSTGt, DLOCt, cmax


def build_inputs(x, edge_index, W1, a_src1, a_dst1, b1, W2, a_src2, a_dst2, b2,
                 n_nodes, nblk, n_cores):
    """Host-side input prep. Returns (in_maps, pos_of_node, bpc, cmax, npad)."""
    bpc = nblk // n_cores
    npad = nblk * P
    src = np.asarray(edge_index[0], dtype=np.int64)
    dst = np.asarray(edge_index[1], dtype=np.int64)
    pos_of_node, SRCt, DSTGt, DLOCt, cmax = preprocess_graph(
        src, dst, n_nodes, nblk)

    x = np.asarray(x, np.float32)
    XTa = np.zeros((IN_DIM, npad), np.float32)
    XTa[:, pos_of_node] = x.T

    W1 = np.asarray(W1, np.float32)
    W2 = np.asarray(W2, np.float32)
    a_src1 = np.asarray(a_src1, np.float32)
    a_dst1 = np.asarray(a_dst1, np.float32)
    a_src2 = np.asarray(a_src2, np.float32)
    a_dst2 = np.asarray(a_dst2, np.float32)
    b1 = np.asarray(b1, np.float32)
    b2 = np.asarray(b2, np.float32)

    A1BDa = np.zeros((HID, 2 * H1), np.float32)
    for h in range(H1):
        A1BDa[h * C1:(h + 1) * C1, h] = a_dst1[h]
        A1BDa[h * C1:(h + 1) * C1, H1 + h] = a_src1[h]
    A2Ta = np.stack([a_dst2[0], a_src2[0]], axis=1).astype(np.float32)

    common = {
        "XT": XTa,
        "W1": W1,
        "W1T": np.ascontiguousarray(W1.T),
        "A1BD": A1BDa,
        "W2": W2,
        "W2T": np.ascontiguousarray(W2.T),
        "A2T": A2Ta,
        "B1R": np.ascontiguousarray(np.broadcast_to(b1, (P, HID))),
        "B2R": np.ascontiguousarray(np.broadcast_to(b2, (P, OUT_DIM))),
        "IOTA": np.ascontiguousarray(
            np.broadcast_to(np.arange(P, dtype=np.float32), (P, P))),
        "IDENT": np.eye(P, dtype=ml_dtypes.bfloat16),
    }
    def flat_meta(a, k):
        # [nblk, P, cmax] core-slice -> device layout [P, bpc*cmax]
        s = a[k * bpc:(k + 1) * bpc]
        return np.ascontiguousarray(
            s.transpose(1, 0, 2).reshape(P, bpc * cmax))

    in_maps = []
    for k in range(n_cores):
        m = dict(common)
        m["SRC"] = flat_meta(SRCt, k)
        m["DSTG"] = flat_meta(DSTGt, k)
        m["DLOC"] = flat_meta(DLOCt, k)
        assert m["SRC"].shape == (P, bpc * cmax)
        in_maps.append(m)
    return in_maps, pos_of_node, bpc, cmax, npad


# ----------------------------------------------------------------------------
# entry point
# ----------------------------------------------------------------------------

_prog_cache = {}
last_results = None  # BassKernelResults of the most recent run (for test.py)


def _get_program(bpc, cmax, npad, n_cores):
    key = (bpc, cmax, npad, n_cores)
    if key not in _prog_cache:
        nc = build_program(bpc, cmax, npad, n_cores)
        # HW-only post-pass (CoreSim rejects the unregistered NoOps)
        _split_excess_waits(nc)
        _prog_cache[key] = nc
    return _prog_cache[key]


def run(inputs, nblk=784, n_cores=N_CORES, n_nodes=N_NODES, trace=False):
    global last_results
    in_maps, pos_of_node, bpc, cmax, npad = build_inputs(
        n_nodes=n_nodes, nblk=nblk, n_cores=n_cores, **inputs)
    nc = _get_program(bpc, cmax, npad, n_cores)
    kwargs = {}
    if trace:
        kwargs = dict(trace=True, trace_cores=list(range(n_cores)),
                      stitch_traces=True)
    res = run_bass_kernel_spmd(
        nc, in_maps, core_ids=list(range(n_cores)), **kwargs)
    last_results = res
    out_all = np.concatenate([r["OUT"] for r in res.results], axis=0)
    return np.ascontiguousarray(out_all[pos_of_node].astype(np.float32))


def kernel(**inputs):
    return run(inputs)
